# revision 3
# baseline (speedup 1.0000x reference)
"""Trainium2 Bass kernel for nn_AFE_78958678770209 (dense_cnn, deformable block).

Pipeline (per sample):
  h   = W1 @ x + b1           (W1 = def_w @ pw1_w @ dw1_w folded on host)
  off = conv3x3(x, poff)      (offsets; bias folded into the base grid)
  g   = bilinear_gather(h, off)
  d2  = conv2x2_s2(g, dw2)    (dw2 bias folded into pw2 bias)
  out = pw2 @ d2 + b_out
Sharding: data-parallel over batch, 32 samples -> 8 cores x 4 samples.

v3 (schedule + instruction-count rework of v2; same gather algorithm):
  - token stripe = [h(s):192 | A(s):18 | z | h(s+1):192+18 junk | z] bf16,
    so ONE strided ACT eviction per block moves both h-pair and the offset
    conv columns (was 2 evictions); taps read A in-place from the stripes.
  - program order per sample: h-blocks -> taps/idx/W -> ALL gather triggers
    -> emit_tail(prev sample) -> combines.  The tail's dw2/pw2 and the next
    sample's h-phase now hide under the ~52us/sample gather DMA window
    (measured floor: 28.7k descriptors x ~116ns / 16 engines).
  - combine: the two v-streams merged into one 3-free-dim op set
    ([128,2,2,NIDX] x-lerp) -> 6 DVE ops per chunk instead of 9.
  - W broadcast evictions split ACT/DVE; 4 SWDGE queues.
"""

import os
import numpy as np
import ml_dtypes

B, C, CO, H, W = 32, 192, 384, 56, 56
PH, PW = H + 2, W + 2              # 58x58 padded raster
NPAD = PH * PW                     # 3364
NPOS = 3456                        # padded to 27*128
NPOS2 = 3584                       # gather stream padded to 4*896
NG = NPOS // 128                   # 27 position blocks
NS = 4                             # samples per core
NCORES = 8
XW = 3584                          # x tile width (>= NPOS+1)
TOKB = 1024                        # token stripe bytes (2 pos x 256 bf16)
HOUT, WOUT = 28, 28
NOUT = HOUT * WOUT                 # 784
UT = 392                           # u-tile: 14 output rows x 28

BF16 = ml_dtypes.bfloat16


def _fold_params(p):
    f32 = np.float32
    W1 = (p['def_w'].astype(f32) @ p['pw1_w'].astype(f32) @ p['dw1_w'].astype(f32))
    b1 = (p['def_w'].astype(f32) @ (p['pw1_w'].astype(f32) @ p['dw1_b'].astype(f32)
                                    + p['pw1_b'].astype(f32)) + p['def_b'].astype(f32))
    b_out = p['pw2_w'].astype(f32) @ p['dw2_b'].astype(f32) + p['pw2_b'].astype(f32)
    return W1, b1, b_out


def _sv(ap2d, boff, h, hstride, w, wstride):
    """Strided [P, h, w] view of a 2-dim AP [P, N] at element offset boff."""
    from bass_rust import AP
    return AP(ap2d.tensor, ap2d.offset + boff,
              [list(ap2d.ap[0]), [hstride, h], [wstride, w]])


def _rap(ap, dims):
    """Raw AP with explicit free dims [(stride, n), ...] at ap's offset."""
    from bass_rust import AP
    return AP(ap.tensor, ap.offset, [list(ap.ap[0])] + [list(d) for d in dims])


def build_nc():
    import concourse.bacc as bacc
    import concourse.mybir as mybir
    import concourse.tile as tile

    NQ = int(os.environ.get('KQUEUES', '4'))
    NIDX = int(os.environ.get('KNIDX', '896'))     # idxs per gather call
    NCH = NPOS2 // (16 * (NIDX // 16))             # chunk count
    assert NPOS2 % NIDX == 0

    nc = bacc.Bacc("TRN2", target_bir_lowering=False, debug=False,
                   num_swdge_queues=NQ)
    dt = mybir.dt
    Alu = mybir.AluOpType
    f32, bf16, i16 = dt.float32, dt.bfloat16, dt.int16

    # ---------------- DRAM parameters ----------------
    x_d = nc.declare_dram_parameter("x", [NS, C + 1, XW], dt.bfloat16, isOutput=False)
    w1st_lo_d = nc.declare_dram_parameter("w1st_lo", [128, C + 18], dt.bfloat16, isOutput=False)
    w1st_hi_d = nc.declare_dram_parameter("w1st_hi", [65, C + 18], dt.bfloat16, isOutput=False)  # row64=[b1|0]
    k2t_d = nc.declare_dram_parameter("k2t", [4, C, C], dt.bfloat16, isOutput=False)       # [t,(c),(o)]
    pw2t_d = nc.declare_dram_parameter("pw2t", [C, CO], dt.bfloat16, isOutput=False)
    bout_d = nc.declare_dram_parameter("bout", [3, 128], dt.float32, isOutput=False)
    base_d = nc.declare_dram_parameter("base", [128, NG, 2], dt.float32, isOutput=False)
    sel_d = nc.declare_dram_parameter("sel", [2, 2, 128], dt.bfloat16, isOutput=False)
    ident_d = nc.declare_dram_parameter("ident", [128, 128], dt.bfloat16, isOutput=False)
    shifts_d = nc.declare_dram_parameter("shifts", [17, 128, 128], dt.bfloat16,
                                         isOutput=False)
    pfold_d = nc.declare_dram_parameter("pfold", [8, 128, 128], dt.float32,
                                        isOutput=False)
    out_d = nc.declare_dram_parameter("out", [NS, CO, NOUT], dt.bfloat16, isOutput=True)

    TAPS = [(t, dy * PW + dx)
            for t, (dy, dx) in enumerate((dy, dx) for dy in (-1, 0, 1)
                                         for dx in (-1, 0, 1))]

    from contextlib import ExitStack
    with ExitStack() as _stk:
        tc = _stk.enter_context(tile.TileContext(nc))
        _p = lambda **kw: _stk.enter_context(tc.tile_pool(**kw))
        cp = _p(name="const", bufs=1)
        xp = _p(name="x", bufs=1)
        ap_ = _p(name="A", bufs=1)
        ip = _p(name="idx", bufs=2)
        wcp = _p(name="wc", bufs=1)
        wp = _p(name="W", bufs=2)
        gp = _p(name="G", bufs=2)
        tp_ = _p(name="tt", bufs=1)
        sp = _p(name="sc", bufs=1)
        gsp = _p(name="gsb", bufs=1)
        d2p = _p(name="d2", bufs=1)
        op = _p(name="osb", bufs=1)
        dp = _p(name="dram", bufs=1, space="DRAM")
        ps = _p(name="ps", bufs=2, space="PSUM")
        pst = _p(name="pst", bufs=1, space="PSUM")
        psA = _p(name="psA", bufs=1, space="PSUM")
        psH = _p(name="psH", bufs=3, space="PSUM")
        if True:
            # x of sample 0 first so its h-phase starts ASAP, then the
            # weights the first matmuls need, then the rest of the consts.
            x_tiles = []
            x_lo0 = xp.tile([128, XW], bf16, tag="xlo")
            x_hi0 = xp.tile([65, XW], bf16, tag="xhi")
            nc.sync.dma_start(x_lo0[:], x_d[0, 0:128, :])
            nc.sync.dma_start(x_hi0[:], x_d[0, 128:193, :])
            w1st_lo = cp.tile([128, C + 18], bf16)
            nc.sync.dma_start(w1st_lo[:], w1st_lo_d[:])
            w1st_hi = cp.tile([65, C + 18], bf16)
            nc.sync.dma_start(w1st_hi[:], w1st_hi_d[:])
            shifts = cp.tile([128, 17, 128], bf16)
            nc.sync.dma_start(shifts[:], shifts_d[:].rearrange("t k m -> k t m"))
            pfold = cp.tile([128, 8, 128], f32)
            nc.sync.dma_start(pfold[:], pfold_d[:].rearrange("t k m -> k t m"))
            base = cp.tile([128, NG, 2], f32)
            nc.sync.dma_start(base[:], base_d[:])
            sel = cp.tile([2, 2, 128], bf16)
            nc.sync.dma_start(sel[:], sel_d[:])
            ident = cp.tile([128, 128], bf16)
            nc.sync.dma_start(ident[:], ident_d[:])
            k2t_lo = cp.tile([128, 4, C], bf16)
            nc.sync.dma_start(k2t_lo[:],
                              k2t_d[:, 0:128, :].rearrange("t c o -> c t o"))
            k2t_hi = cp.tile([64, 4, C], bf16)
            nc.sync.dma_start(k2t_hi[:],
                              k2t_d[:, 128:192, :].rearrange("t c o -> c t o"))
            pw2t_lo = cp.tile([128, CO], bf16)
            nc.sync.dma_start(pw2t_lo[:], pw2t_d[0:128, :])
            pw2t_hi = cp.tile([64, CO], bf16)
            nc.sync.dma_start(pw2t_hi[:], pw2t_d[128:192, :])
            bout = cp.tile([128, 3], f32)
            nc.sync.dma_start(bout[:], bout_d[:].rearrange("b p -> p b"))

            from concourse import library_config
            nc.gpsimd.load_library(library_config.mlp)

            # token pair buffers (manual ping-pong); stripe =
            # [h(s):192 | A(s):18 | z:46 | h(s+1)+junk:210 | z:46] bf16.
            # Group NG is a permanent zero pad for the shifted tap views.
            tok_bufs = []
            for _tb in range(2):
                _tok = cp.tile([128, NG + 1, 512], bf16, tag=f"tokbuf{_tb}")
                nc.vector.memset(_rap(_tok[:, 0, 0], [(1, (NG + 1) * 512)]), 0.0)
                tok_bufs.append(_tok)

            def emit_tail(si, g_sb):
                # ---------------- dw2 (2x2 stride-2) -------------------------
                d2_lo = d2p.tile([128, NOUT], bf16, tag="d2lo")
                d2_hi = d2p.tile([64, NOUT], bf16, tag="d2hi")
                glo = g_sb[:, 0, :]
                ghi = g_sb[0:64, 1, :]
                for obase, osz, dtile in ((0, 128, d2_lo), (128, 64, d2_hi)):
                    for ut in range(2):
                        pd = ps.tile([osz, UT], f32, tag="ps")
                        for t in range(4):
                            dy, dx = t // 2, t % 2
                            boff = PW * (1 + dy) + (1 + dx) + ut * 14 * 2 * PW
                            rhs_lo = _sv(glo, boff, 14, 2 * PW, 28, 2)
                            rhs_hi = _sv(ghi, boff, 14, 2 * PW, 28, 2)
                            nc.tensor.matmul(
                                pd[:], k2t_lo[:, t, obase:obase + osz], rhs_lo,
                                start=(t == 0), stop=False)
                            nc.tensor.matmul(
                                pd[:], k2t_hi[:, t, obase:obase + osz], rhs_hi,
                                start=False, stop=(t == 3))
                        nc.scalar.copy(dtile[:, ut * UT:(ut + 1) * UT], pd[:])
                # ---------------- pw2 ----------------------------------------
                out_sb = op.tile([128, 3, NOUT], bf16, tag="osb")
                for o3 in range(3):
                    osl = slice(o3 * 128, (o3 + 1) * 128)
                    for ut in range(2):
                        usl = slice(ut * UT, (ut + 1) * UT)
                        po = ps.tile([128, UT], f32, tag="ps")
                        nc.tensor.matmul(po[:], pw2t_lo[:, osl],
                                         d2_lo[:, usl], start=True, stop=False)
                        nc.tensor.matmul(po[:], pw2t_hi[:, osl],
                                         d2_hi[:, usl], start=False, stop=True)
                        nc.scalar.add(out_sb[:, o3, usl], po[:],
                                      bout[:, o3:o3 + 1])
                nc.sync.dma_start(
                    out_d[si, :, :].rearrange("(b p) n -> p b n", p=128),
                    out_sb[:])

            prev = None   # (si, g_sb) of the previous sample
            for si in range(NS):
                # ---------------- load x ----------------
                if si == 0:
                    x_lo, x_hi = x_lo0, x_hi0
                else:
                    x_lo = xp.tile([128, XW], bf16, tag="xlo")
                    x_hi = xp.tile([65, XW], bf16, tag="xhi")
                    nc.sync.dma_start(x_lo[:], x_d[si, 0:128, :])
                    nc.sync.dma_start(x_hi[:], x_d[si, 128:193, :])

                # ------- h pair-tokens + A columns, one eviction/block -------
                tok = tok_bufs[si % 2]
                for g in range(NG):
                    s0 = g * 128
                    ph = psH.tile([128, 420], f32, tag="psH")
                    nc.tensor.matmul(ph[:, 0:210], x_lo[:, s0:s0 + 128],
                                     w1st_lo[:], start=True, stop=False)
                    nc.tensor.matmul(ph[:, 0:210], x_hi[:, s0:s0 + 128],
                                     w1st_hi[:], start=False, stop=True)
                    nc.tensor.matmul(ph[:, 210:420],
                                     x_lo[:, s0 + 1:s0 + 129],
                                     w1st_lo[:], start=True, stop=False)
                    nc.tensor.matmul(ph[:, 210:420],
                                     x_hi[:, s0 + 1:s0 + 129],
                                     w1st_hi[:], start=False, stop=True)
                    # tok[p, g, {0:210, 256:466}] = [h|A](s), [h|A](s+1)
                    nc.scalar.copy(
                        _rap(tok[:, g, 0], [(256, 2), (1, 210)]),
                        _rap(ph[:, 0], [(210, 2), (1, 210)]))

                # ------- 9-tap shifted sum via PE one-hot shift matmuls -------
                # A(s) lives at stripe elems [192,210) of tok slot 0.
                def Arhs(g0, n, co):
                    return _rap(tok[:, g0, 192 + co], [(512, n), (1, 2)])
                ps_off = psA.tile([128, 54], f32, tag="psOff")
                ofull = _rap(ps_off[:, 0], [(2, NG), (1, 2)])
                nc.tensor.matmul(ofull, shifts[:, 0, :], Arhs(0, NG, 8),
                                 start=True, stop=False)
                mi = 1
                for t, d in TAPS:
                    if d == 0:
                        continue
                    co = 2 * t
                    nc.tensor.matmul(ofull, shifts[:, mi, :], Arhs(0, NG, co),
                                     start=False, stop=False)
                    last = (mi + 1 == 16)
                    if d > 0:
                        nc.tensor.matmul(ofull, shifts[:, mi + 1, :],
                                         Arhs(1, NG, co),
                                         start=False, stop=last)
                    else:
                        nc.tensor.matmul(_rap(ps_off[:, 2], [(2, NG - 1), (1, 2)]),
                                         shifts[:, mi + 1, :],
                                         Arhs(0, NG - 1, co),
                                         start=False, stop=last)
                    mi += 2
                acc = ap_.tile([128, NG, 2], f32, tag="acc")
                nc.vector.tensor_copy(acc[:], ofull)

                # ---------------- index math ----------------
                pyx = ap_.tile([128, NG, 2], f32, tag="pyx")
                nc.vector.tensor_tensor(pyx[:], acc[:], base[:], Alu.add)
                nc.vector.tensor_scalar(pyx[:], pyx[:], 0.0, float(H - 1),
                                        Alu.max, Alu.min)
                y0i = ap_.tile([128, NG, 2], dt.int32, tag="y0i")
                nc.vector.tensor_copy(y0i[:], pyx[:])
                icast = ap_.tile([128, NG, 2], f32, tag="icast")
                nc.vector.tensor_copy(icast[:], y0i[:])
                gtt = ap_.tile([128, NG, 2], f32, tag="gtt")
                nc.vector.tensor_tensor(gtt[:], icast[:], pyx[:], Alu.is_gt)
                ifl = ap_.tile([128, NG, 2], f32, tag="ifl")
                nc.vector.tensor_tensor(ifl[:], icast[:], gtt[:], Alu.subtract)
                frac = ap_.tile([128, NG, 2], f32, tag="frac")
                nc.vector.tensor_tensor(frac[:], pyx[:], ifl[:], Alu.subtract)
                # token base id: Bt = 59 + 58*ifl_y + ifl_x
                Bt = ap_.tile([128, NG], f32, tag="Bt")
                nc.vector.tensor_scalar(Bt[:], ifl[:, :, 0], float(PW),
                                        float(PW + 1), Alu.mult, Alu.add)
                nc.vector.tensor_tensor(Bt[:], Bt[:], ifl[:, :, 1], Alu.add)
                # 16-wrap fold via PE one-hot matmuls (fp32, exact ints):
                # ps_idx[16j+q, 8g+m] = Bt[16m+q, g] for all j
                ps_idx = pst.tile([128, 224], f32, tag="pidx")
                for m in range(8):
                    nc.tensor.matmul(_rap(ps_idx[:, m], [(8, NG)]),
                                     pfold[:, m, :], Bt[:],
                                     start=True, stop=True)
                idxs = ip.tile([128, 2, 224], i16, tag="idxs")
                nc.vector.memset(idxs[:, :, 216:224], 0)
                nc.vector.tensor_scalar(idxs[:, 0, 0:216], ps_idx[:, 0:216],
                                        0.0, None, Alu.add)
                nc.vector.tensor_scalar(idxs[:, 1, 0:216], ps_idx[:, 0:216],
                                        float(PW), None, Alu.add)

                # ---------------- wx/wy broadcast ----------------
                w_bf = ip.tile([128, 2, NG], bf16, tag="wbf")   # [p, o, g]
                nc.vector.tensor_copy(
                    w_bf[:], _rap(frac[:, 0, 0], [(1, 2), (2, NG)]))
                ps_t = pst.tile([54, 128], bf16, tag="pst")
                nc.tensor.transpose(ps_t[:], _rap(w_bf[:, 0, 0], [(1, 54)]),
                                    ident[:])
                w_cols = wcp.tile([54, 128], bf16, tag="wcols")
                nc.scalar.copy(w_cols[:], ps_t[:])
                # HBM bounce: [54,128] col-major -> [2, NPOS] row layout
                w_hbm = dp.tile([54, 128], bf16, tag="whbm")
                nc.sync.dma_start(w_hbm[:], w_cols[:])
                w_rows = wcp.tile([2, NPOS2], bf16, tag="wrows")
                nc.vector.memset(w_rows[:, NPOS:NPOS2], 0.0)
                nc.sync.dma_start(
                    w_rows[:, 0:NPOS],
                    w_hbm[:].rearrange("(o g) p -> o (g p)", o=2))
                WY = wp.tile([128, NPOS2], bf16, tag="WY")
                WX = wp.tile([128, NPOS2], bf16, tag="WX")
                for o, Wt, eng in ((0, WY, nc.scalar), (1, WX, nc.vector)):
                    for n0 in range(0, NPOS2, 512):
                        n1 = min(n0 + 512, NPOS2)
                        pwt = ps.tile([128, 512], f32, tag="ps")
                        nc.tensor.matmul(pwt[:, 0:n1 - n0], sel[:, o, :],
                                         w_rows[:, n0:n1],
                                         start=True, stop=True)
                        if eng is nc.scalar:
                            nc.scalar.copy(Wt[:, n0:n1], pwt[:, 0:n1 - n0])
                        else:
                            nc.vector.tensor_copy(Wt[:, n0:n1], pwt[:, 0:n1 - n0])

                # ------- gather triggers + combines, software-pipelined ------
                # Gt pool has 2 bufs: trigger(c) may only be emitted after
                # combine(c-2); g_sb (1 buf) may only be allocated after
                # emit_tail(prev) read the previous sample's tile.
                NI16 = NIDX // 16
                tok_flat = _rap(tok[:, 0, 0], [(1, NG * 512)])
                Gts = {}
                g_sb = None

                def trigger(c2):
                    Gt = gp.tile([128, 8, NIDX], bf16, tag="G")
                    for v in range(2):
                        nc.gpsimd.dma_gather(
                            Gt[:, 4 * v:4 * (v + 1), :],
                            tok_flat,
                            idxs[:, v, c2 * NI16:(c2 + 1) * NI16],
                            num_idxs=NIDX, num_idxs_reg=NIDX,
                            elem_size=512, transpose=True,
                            queue_num=(c2 * 2 + v) % NQ,
                            sbuf_tokens_per_rank=128,
                            sbuf_free_dim_per_rank=TOKB)
                    Gts[c2] = Gt

                def combine(c2):
                    Gt = Gts.pop(c2)
                    n0 = c2 * NIDX
                    # corners: [v0: L(2) R(2) | v1: L(2) R(2)] groups of NIDX
                    L = _rap(Gt[:, 0, 0], [(4 * NIDX, 2), (NIDX, 2), (1, NIDX)])
                    R = _rap(Gt[:, 2, 0], [(4 * NIDX, 2), (NIDX, 2), (1, NIDX)])
                    WXc = _rap(WX[:, n0], [(0, 2), (0, 2), (1, NIDX)])
                    WYc = _rap(WY[:, n0], [(0, 2), (1, NIDX)])
                    tt = tp_.tile([128, 2, 2, NIDX], bf16, tag="tt")
                    dd = sp.tile([128, 2, 2, NIDX], bf16, tag="dd")
                    mm = sp.tile([128, 2, 2, NIDX], bf16, tag="mm")
                    nc.vector.tensor_tensor(dd[:], R, L, Alu.subtract)
                    nc.vector.tensor_tensor(mm[:], dd[:], WXc, Alu.mult)
                    nc.vector.tensor_tensor(tt[:], L, mm[:], Alu.add)
                    # y-lerp
                    dy_ = sp.tile([128, 2, NIDX], bf16, tag="dy")
                    nc.vector.tensor_tensor(dy_[:], tt[:, 1, :, :],
                                            tt[:, 0, :, :], Alu.subtract)
                    my_ = sp.tile([128, 2, NIDX], bf16, tag="my")
                    nc.vector.tensor_tensor(my_[:], dy_[:], WYc, Alu.mult)
                    nc.vector.tensor_tensor(
                        _rap(g_sb[:, 0, n0], [(NPOS2, 2), (1, NIDX)]),
                        tt[:, 0, :, :], my_[:], Alu.add)

                trigger(0)
                if NCH > 1:
                    trigger(1)
                # dw2/pw2/store of the previous sample overlaps the gather DMA
                if prev is not None:
                    emit_tail(*prev)
                g_sb = gsp.tile([128, 2, NPOS2], bf16, tag="gsb")
                for c2 in range(2, NCH):
                    combine(c2 - 2)
                    trigger(c2)
                for c2 in range(max(0, NCH - 2), NCH):
                    combine(c2)
                prev = (si, g_sb)

            emit_tail(*prev)

    nc.compile()
    return nc


def _prep_inputs(p):
    x = p['x'].astype(np.float32)
    W1, b1, b_out = _fold_params(p)

    xpad = np.zeros((B, C, PH, PW), np.float32)
    xpad[:, :, 1:PH - 1, 1:PW - 1] = x
    xflat = np.zeros((B, C + 1, XW), np.float32)
    xflat[:, 0:C, 0:NPAD] = xpad.reshape(B, C, NPAD)
    xflat[:, C, :] = 1.0
    xflat = xflat.astype(BF16)

    poff = p['poff_w'].astype(np.float32)          # [2, C, 3, 3]
    wst = np.zeros((C, 18), np.float32)            # col = t*2 + o
    for t in range(9):
        dy, dx = t // 3, t % 3
        for o in range(2):
            wst[:, t * 2 + o] = poff[o, :, dy, dx]
    w1st = np.concatenate([W1.T, wst], axis=1)     # [C, 210]
    w1st_lo = np.ascontiguousarray(w1st[0:128]).astype(BF16)
    w1st_hi = np.zeros((65, C + 18), np.float32)
    w1st_hi[0:64] = w1st[128:192]
    w1st_hi[64, 0:C] = b1
    w1st_hi = w1st_hi.astype(BF16)

    dw2 = p['dw2_w'].astype(np.float32)            # [O, C, 2, 2]
    k2t = np.zeros((4, C, C), np.float32)
    for t in range(4):
        dy, dx = t // 2, t % 2
        k2t[t] = dw2[:, :, dy, dx].T               # [c, o]
    k2t = k2t.astype(BF16)

    pw2t = np.ascontiguousarray(p['pw2_w'].astype(np.float32).T).astype(BF16)
    bout = b_out.reshape(3, 128).astype(np.float32)

    s = np.arange(NPOS, dtype=np.float32)
    ypad = np.floor_divide(np.minimum(s, NPAD - 1), PW)
    xpad_c = np.minimum(s, NPAD - 1) % PW
    base = np.zeros((128, NG, 2), np.float32)
    base[:, :, 0] = (ypad - 1.0 + float(p['poff_b'][0])).reshape(NG, 128).T
    base[:, :, 1] = (xpad_c - 1.0 + float(p['poff_b'][1])).reshape(NG, 128).T

    sel = np.zeros((2, 2, 128), np.float32)
    sel[0, 0, :] = 1.0
    sel[1, 1, :] = 1.0
    sel = sel.astype(BF16)
    ident = np.eye(128, dtype=np.float32).astype(BF16)

    # one-hot shift matrices for the 9-tap sum: out[m] += A[m + d] via
    # lhsT[k, m] = 1 at k = m+d (main) / k = m+d-+128 (block-crossing wrap)
    shifts = np.zeros((17, 128, 128), np.float32)
    shifts[0] = np.eye(128)
    mi = 1
    for t in range(9):
        d = (t // 3 - 1) * PW + (t % 3 - 1)
        if d == 0:
            continue
        for m in range(128):
            k = m + d
            if 0 <= k < 128:
                shifts[mi, k, m] = 1.0
            kw = m + d - 128 if d > 0 else m + d + 128
            if 0 <= kw < 128:
                shifts[mi + 1, kw, m] = 1.0
        mi += 2
    shifts = shifts.astype(BF16)

    # fold matrices: ps_idx[16j+q, 8g+m] = Bt[16m+q, g]
    pfold = np.zeros((8, 128, 128), np.float32)
    for m in range(8):
        for mp in range(128):
            pfold[m, 16 * m + (mp % 16), mp] = 1.0

    shared = dict(w1st_lo=w1st_lo, w1st_hi=w1st_hi,
                  k2t=k2t, pw2t=pw2t, bout=bout, base=base, sel=sel,
                  ident=ident, shifts=shifts, pfold=pfold)
    in_maps = []
    for ci in range(NCORES):
        m = dict(shared)
        m['x'] = np.ascontiguousarray(xflat[ci * NS:(ci + 1) * NS])
        in_maps.append(m)
    return in_maps


def kernel(**inputs):
    from concourse.bass_utils import run_bass_kernel_spmd

    p = {k: np.asarray(v) for k, v in inputs.items()}
    in_maps = _prep_inputs(p)
    nc = build_nc()
    res = run_bass_kernel_spmd(nc, in_maps, core_ids=list(range(NCORES)))
    outs = [res.results[ci]['out'] for ci in range(NCORES)]
    out = np.concatenate([np.asarray(o).astype(np.float32) for o in outs],
                         axis=0)
    return out.reshape(B, CO, HOUT, WOUT)


# revision 13
# speedup vs baseline: 1.2730x; 1.2730x over previous
"""Trainium2 Bass kernel for nn_AFE_78958678770209 (dense_cnn, deformable block).

Pipeline (per sample):
  h   = W1 @ x + b1           (W1 = def_w @ pw1_w @ dw1_w folded on host)
  off = conv3x3(x, poff)      (offsets; bias folded into the base grid)
  g   = bilinear_gather(h, off)
  d2  = conv2x2_s2(g, dw2)    (dw2 bias folded into pw2 bias)
  out = pw2 @ d2 + b_out
Sharding: data-parallel over batch, 32 samples -> 8 cores x 4 samples.

v3 (schedule + instruction-count rework of v2; same gather algorithm):
  - token stripe = [h(s):192 | A(s):18 | z | h(s+1):192+18 junk | z] bf16,
    so ONE strided ACT eviction per block moves both h-pair and the offset
    conv columns (was 2 evictions); taps read A in-place from the stripes.
  - program order per sample: h-blocks -> taps/idx/W -> ALL gather triggers
    -> emit_tail(prev sample) -> combines.  The tail's dw2/pw2 and the next
    sample's h-phase now hide under the ~52us/sample gather DMA window
    (measured floor: 28.7k descriptors x ~116ns / 16 engines).
  - combine: the two v-streams merged into one 3-free-dim op set
    ([128,2,2,NIDX] x-lerp) -> 6 DVE ops per chunk instead of 9.
  - W broadcast evictions split ACT/DVE; 4 SWDGE queues.
"""

import os
import numpy as np
import ml_dtypes

B, C, CO, H, W = 32, 192, 384, 56, 56
PH, PW = H + 2, W + 2              # 58x58 padded raster
NPAD = PH * PW                     # 3364
NPOS = 3456                        # padded to 27*128
NPOS2 = 3584                       # gather stream padded to 4*896
NG = NPOS // 128                   # 27 position blocks
NS = 4                             # samples per core
NCORES = 8
XW = 3584                          # x tile width (>= NPOS+1)
TOKB = 1024                        # token stripe bytes (2 pos x 256 bf16)
HOUT, WOUT = 28, 28
NOUT = HOUT * WOUT                 # 784
UT = 392                           # u-tile: 14 output rows x 28

BF16 = ml_dtypes.bfloat16


def _fold_params(p):
    f32 = np.float32
    W1 = (p['def_w'].astype(f32) @ p['pw1_w'].astype(f32) @ p['dw1_w'].astype(f32))
    b1 = (p['def_w'].astype(f32) @ (p['pw1_w'].astype(f32) @ p['dw1_b'].astype(f32)
                                    + p['pw1_b'].astype(f32)) + p['def_b'].astype(f32))
    b_out = p['pw2_w'].astype(f32) @ p['dw2_b'].astype(f32) + p['pw2_b'].astype(f32)
    return W1, b1, b_out


def _sv(ap2d, boff, h, hstride, w, wstride):
    """Strided [P, h, w] view of a 2-dim AP [P, N] at element offset boff."""
    from bass_rust import AP
    return AP(ap2d.tensor, ap2d.offset + boff,
              [list(ap2d.ap[0]), [hstride, h], [wstride, w]])


def _rap(ap, dims):
    """Raw AP with explicit free dims [(stride, n), ...] at ap's offset."""
    from bass_rust import AP
    return AP(ap.tensor, ap.offset, [list(ap.ap[0])] + [list(d) for d in dims])


def build_nc():
    import concourse.bacc as bacc
    import concourse.mybir as mybir
    import concourse.tile as tile

    NQ = int(os.environ.get('KQUEUES', '1'))
    # Uniform chunks (single Gt tag, 3 bufs).  The LAST combine of each
    # sample is deferred into the next iteration, after the next sample's
    # idx math, so the DVE FIFO never holds up the next gather triggers.
    CHUNKS = (896, 896, 896, 896)
    assert sum(CHUNKS) == NPOS2 and all(c % 128 == 0 for c in CHUNKS)
    NCH = len(CHUNKS)
    CH0 = [sum(CHUNKS[:i]) for i in range(NCH)]    # start offsets

    nc = bacc.Bacc("TRN2", target_bir_lowering=False, debug=False,
                   num_swdge_queues=NQ)
    dt = mybir.dt
    Alu = mybir.AluOpType
    f32, bf16, i16 = dt.float32, dt.bfloat16, dt.int16

    # ---------------- DRAM parameters ----------------
    x_d = nc.declare_dram_parameter("x", [NS, C + 1, XW], dt.bfloat16, isOutput=False)
    w1st_lo_d = nc.declare_dram_parameter("w1st_lo", [128, C + 18], dt.bfloat16, isOutput=False)
    w1st_hi_d = nc.declare_dram_parameter("w1st_hi", [65, C + 18], dt.bfloat16, isOutput=False)  # row64=[b1|0]
    k2t_d = nc.declare_dram_parameter("k2t", [4, C, C], dt.bfloat16, isOutput=False)       # [t,(c),(o)]
    pw2t_d = nc.declare_dram_parameter("pw2t", [C, CO], dt.bfloat16, isOutput=False)
    bout_d = nc.declare_dram_parameter("bout", [3, 128], dt.float32, isOutput=False)
    base_d = nc.declare_dram_parameter("base", [128, NG, 2], dt.float32, isOutput=False)
    sel_d = nc.declare_dram_parameter("sel", [2, 2, 128], dt.bfloat16, isOutput=False)
    ident_d = nc.declare_dram_parameter("ident", [128, 128], dt.bfloat16, isOutput=False)
    shifts_d = nc.declare_dram_parameter("shifts", [17, 128, 128], dt.bfloat16,
                                         isOutput=False)
    pfold_d = nc.declare_dram_parameter("pfold", [8, 128, 128], dt.float32,
                                        isOutput=False)
    out_d = nc.declare_dram_parameter("out", [NS, CO, NOUT], dt.bfloat16, isOutput=True)

    TAPS = [(t, dy * PW + dx)
            for t, (dy, dx) in enumerate((dy, dx) for dy in (-1, 0, 1)
                                         for dx in (-1, 0, 1))]

    from contextlib import ExitStack
    with ExitStack() as _stk:
        tc = _stk.enter_context(tile.TileContext(nc))
        _p = lambda **kw: _stk.enter_context(tc.tile_pool(**kw))
        cp = _p(name="const", bufs=1)
        xp = _p(name="x", bufs=1)
        ap_ = _p(name="A", bufs=1)
        ip = _p(name="idx", bufs=2)
        wcp = _p(name="wc", bufs=1)
        wp = _p(name="W", bufs=2)
        gp = _p(name="G", bufs=4)
        tp_ = _p(name="tt", bufs=1)
        sp = _p(name="sc", bufs=1)
        gsp = _p(name="gsb", bufs=1)
        d2p = _p(name="d2", bufs=1)
        op = _p(name="osb", bufs=1)
        dp = _p(name="dram", bufs=1, space="DRAM")
        ps = _p(name="ps", bufs=2, space="PSUM")
        pst = _p(name="pst", bufs=1, space="PSUM")
        psA = _p(name="psA", bufs=1, space="PSUM")
        psH = _p(name="psH", bufs=3, space="PSUM")
        if True:
            # x of sample 0 first so its h-phase starts ASAP, then the
            # weights the first matmuls need, then the rest of the consts.
            x_tiles = []
            x_lo0 = xp.tile([128, XW], bf16, tag="xlo")
            x_hi0 = xp.tile([65, XW], bf16, tag="xhi")
            nc.sync.dma_start(x_lo0[:], x_d[0, 0:128, :])
            nc.sync.dma_start(x_hi0[:], x_d[0, 128:193, :])
            w1st_lo = cp.tile([128, C + 18], bf16)
            nc.sync.dma_start(w1st_lo[:], w1st_lo_d[:])
            w1st_hi = cp.tile([65, C + 18], bf16)
            nc.sync.dma_start(w1st_hi[:], w1st_hi_d[:])
            shifts = cp.tile([128, 17, 128], bf16)
            nc.sync.dma_start(shifts[:], shifts_d[:].rearrange("t k m -> k t m"))
            pfold = cp.tile([128, 8, 128], f32)
            nc.sync.dma_start(pfold[:], pfold_d[:].rearrange("t k m -> k t m"))
            base = cp.tile([128, NG, 2], f32)
            nc.sync.dma_start(base[:], base_d[:])
            sel = cp.tile([2, 2, 128], bf16)
            nc.sync.dma_start(sel[:], sel_d[:])
            ident = cp.tile([128, 128], bf16)
            nc.sync.dma_start(ident[:], ident_d[:])
            k2t_lo = cp.tile([128, 4, C], bf16)
            nc.sync.dma_start(k2t_lo[:],
                              k2t_d[:, 0:128, :].rearrange("t c o -> c t o"))
            k2t_hi = cp.tile([64, 4, C], bf16)
            nc.sync.dma_start(k2t_hi[:],
                              k2t_d[:, 128:192, :].rearrange("t c o -> c t o"))
            pw2t_lo = cp.tile([128, CO], bf16)
            nc.sync.dma_start(pw2t_lo[:], pw2t_d[0:128, :])
            pw2t_hi = cp.tile([64, CO], bf16)
            nc.sync.dma_start(pw2t_hi[:], pw2t_d[128:192, :])
            bout = cp.tile([128, 3], f32)
            nc.sync.dma_start(bout[:], bout_d[:].rearrange("b p -> p b"))

            from concourse import library_config
            nc.gpsimd.load_library(library_config.mlp)

            # token pair buffers (manual ping-pong); stripe =
            # [h(s):192 | A(s):18 | z:46 | h(s+1)+junk:210 | z:46] bf16.
            # Group NG is a permanent zero pad for the shifted tap views.
            tok_bufs = []
            for _tb in range(2):
                _tok = cp.tile([128, NG + 1, 512], bf16, tag=f"tokbuf{_tb}")
                # zero only what must be zero: the tap-pad group NG and the
                # two z-regions of every stripe (cheap; the big full-tile
                # memset serialized ~20us of DVE time at startup).
                nc.vector.memset(_tok[:, NG, :], 0.0)
                nc.vector.memset(_rap(_tok[:, 0, 210], [(512, NG), (1, 46)]), 0.0)
                nc.vector.memset(_rap(_tok[:, 0, 466], [(512, NG), (1, 46)]), 0.0)
                tok_bufs.append(_tok)

            def emit_tail(si, g_sb):
                # ---------------- dw2 (2x2 stride-2) -------------------------
                d2_lo = d2p.tile([128, NOUT], bf16, tag="d2lo")
                d2_hi = d2p.tile([64, NOUT], bf16, tag="d2hi")
                glo = g_sb[:, 0, :]
                ghi = g_sb[0:64, 1, :]
                for obase, osz, dtile in ((0, 128, d2_lo), (128, 64, d2_hi)):
                    for ut in range(2):
                        pd = ps.tile([osz, UT], f32, tag="ps")
                        for t in range(4):
                            dy, dx = t // 2, t % 2
                            boff = PW * (1 + dy) + (1 + dx) + ut * 14 * 2 * PW
                            rhs_lo = _sv(glo, boff, 14, 2 * PW, 28, 2)
                            rhs_hi = _sv(ghi, boff, 14, 2 * PW, 28, 2)
                            nc.tensor.matmul(
                                pd[:], k2t_lo[:, t, obase:obase + osz], rhs_lo,
                                start=(t == 0), stop=False)
                            nc.tensor.matmul(
                                pd[:], k2t_hi[:, t, obase:obase + osz], rhs_hi,
                                start=False, stop=(t == 3))
                        nc.scalar.copy(dtile[:, ut * UT:(ut + 1) * UT], pd[:])
                # ---------------- pw2 ----------------------------------------
                out_sb = op.tile([128, 3, NOUT], bf16, tag="osb")
                for o3 in range(3):
                    osl = slice(o3 * 128, (o3 + 1) * 128)
                    for ut in range(2):
                        usl = slice(ut * UT, (ut + 1) * UT)
                        po = ps.tile([128, UT], f32, tag="ps")
                        nc.tensor.matmul(po[:], pw2t_lo[:, osl],
                                         d2_lo[:, usl], start=True, stop=False)
                        nc.tensor.matmul(po[:], pw2t_hi[:, osl],
                                         d2_hi[:, usl], start=False, stop=True)
                        nc.scalar.add(out_sb[:, o3, usl], po[:],
                                      bout[:, o3:o3 + 1])
                nc.sync.dma_start(
                    out_d[si, :, :].rearrange("(b p) n -> p b n", p=128),
                    out_sb[:])

            prev = None   # (si, g_sb) of the previous sample
            pend = None   # deferred last-chunk combine of the previous sample
            xt = (x_lo0, x_hi0)
            for si in range(NS):
                x_lo, x_hi = xt

                # ------- h pair-tokens + A columns, one eviction/block -------
                tok = tok_bufs[si % 2]
                for g in range(NG):
                    s0 = g * 128
                    ph = psH.tile([128, 420], f32, tag="psH")
                    nc.tensor.matmul(ph[:, 0:210], x_lo[:, s0:s0 + 128],
                                     w1st_lo[:], start=True, stop=False)
                    nc.tensor.matmul(ph[:, 0:210], x_hi[:, s0:s0 + 128],
                                     w1st_hi[:], start=False, stop=True)
                    nc.tensor.matmul(ph[:, 210:420],
                                     x_lo[:, s0 + 1:s0 + 129],
                                     w1st_lo[:], start=True, stop=False)
                    nc.tensor.matmul(ph[:, 210:420],
                                     x_hi[:, s0 + 1:s0 + 129],
                                     w1st_hi[:], start=False, stop=True)
                    # tok[p, g, {0:210, 256:466}] = [h|A](s), [h|A](s+1)
                    nc.scalar.copy(
                        _rap(tok[:, g, 0], [(256, 2), (1, 210)]),
                        _rap(ph[:, 0], [(210, 2), (1, 210)]))

                # ------- 9-tap shifted sum via PE one-hot shift matmuls -------
                # A(s) lives at stripe elems [192,210) of tok slot 0.
                def Arhs(g0, n, co):
                    return _rap(tok[:, g0, 192 + co], [(512, n), (1, 2)])
                ps_off = psA.tile([128, 54], f32, tag="psOff")
                ofull = _rap(ps_off[:, 0], [(2, NG), (1, 2)])
                nc.tensor.matmul(ofull, shifts[:, 0, :], Arhs(0, NG, 8),
                                 start=True, stop=False)
                mi = 1
                for t, d in TAPS:
                    if d == 0:
                        continue
                    co = 2 * t
                    nc.tensor.matmul(ofull, shifts[:, mi, :], Arhs(0, NG, co),
                                     start=False, stop=False)
                    last = (mi + 1 == 16)
                    if d > 0:
                        nc.tensor.matmul(ofull, shifts[:, mi + 1, :],
                                         Arhs(1, NG, co),
                                         start=False, stop=last)
                    else:
                        nc.tensor.matmul(_rap(ps_off[:, 2], [(2, NG - 1), (1, 2)]),
                                         shifts[:, mi + 1, :],
                                         Arhs(0, NG - 1, co),
                                         start=False, stop=last)
                    mi += 2
                acc = ap_.tile([128, NG, 2], f32, tag="acc")
                nc.vector.tensor_copy(acc[:], ofull)

                # ---------------- index math ----------------
                pyx = ap_.tile([128, NG, 2], f32, tag="pyx")
                nc.vector.tensor_tensor(pyx[:], acc[:], base[:], Alu.add)
                nc.vector.tensor_scalar(pyx[:], pyx[:], 0.0, float(H - 1),
                                        Alu.max, Alu.min)
                y0i = ap_.tile([128, NG, 2], dt.int32, tag="y0i")
                nc.vector.tensor_copy(y0i[:], pyx[:])
                icast = ap_.tile([128, NG, 2], f32, tag="icast")
                nc.vector.tensor_copy(icast[:], y0i[:])
                gtt = ap_.tile([128, NG, 2], f32, tag="gtt")
                nc.vector.tensor_tensor(gtt[:], icast[:], pyx[:], Alu.is_gt)
                ifl = ap_.tile([128, NG, 2], f32, tag="ifl")
                nc.vector.tensor_tensor(ifl[:], icast[:], gtt[:], Alu.subtract)
                frac = ap_.tile([128, NG, 2], f32, tag="frac")
                nc.vector.tensor_tensor(frac[:], pyx[:], ifl[:], Alu.subtract)
                # token base id: Bt = 59 + 58*ifl_y + ifl_x
                Bt = ap_.tile([128, NG], f32, tag="Bt")
                nc.vector.tensor_scalar(Bt[:], ifl[:, :, 0], float(PW),
                                        float(PW + 1), Alu.mult, Alu.add)
                nc.vector.tensor_tensor(Bt[:], Bt[:], ifl[:, :, 1], Alu.add)
                # 16-wrap fold via PE one-hot matmuls (fp32, exact ints):
                # ps_idx[16j+q, 8g+m] = Bt[16m+q, g] for all j
                ps_idx = pst.tile([128, 224], f32, tag="pidx")
                for m in range(8):
                    nc.tensor.matmul(_rap(ps_idx[:, m], [(8, NG)]),
                                     pfold[:, m, :], Bt[:],
                                     start=True, stop=True)
                idxs = ip.tile([128, 2, 224], i16, tag="idxs")
                nc.vector.memset(idxs[:, :, 216:224], 0)
                nc.vector.tensor_scalar(idxs[:, 0, 0:216], ps_idx[:, 0:216],
                                        0.0, None, Alu.add)
                nc.vector.tensor_scalar(idxs[:, 1, 0:216], ps_idx[:, 0:216],
                                        float(PW), None, Alu.add)

                # ------- gather triggers + combines, software-pipelined ------
                tok_flat = _rap(tok[:, 0, 0], [(1, NG * 512)])

                def trigger(c2):
                    n0, n = CH0[c2], CHUNKS[c2]
                    Gt = gp.tile([128, 8, n], bf16, tag="G")
                    for v in range(2):
                        nc.gpsimd.dma_gather(
                            Gt[:, 4 * v:4 * (v + 1), :],
                            tok_flat,
                            idxs[:, v, n0 // 16:(n0 + n) // 16],
                            num_idxs=n, num_idxs_reg=n,
                            elem_size=512, transpose=True,
                            queue_num=(c2 * 2 + v) % NQ,
                            sbuf_tokens_per_rank=128,
                            sbuf_free_dim_per_rank=TOKB)
                    return Gt

                def combine(ctx):
                    Gt, WYt, WXt, gsb_t, c2 = ctx
                    n0, n = CH0[c2], CHUNKS[c2]
                    # corners: [v0: L(2) R(2) | v1: L(2) R(2)] groups of n.
                    # All lerps run in-place inside Gt (no scratch tiles):
                    #   R := (R - L) * wx;  L := L + R   (x-lerp, both v)
                    #   t1 := (t1 - t0) * wy; out := t0 + t1   (y-lerp)
                    L = _rap(Gt[:, 0, 0], [(4 * n, 2), (n, 2), (1, n)])
                    R = _rap(Gt[:, 2, 0], [(4 * n, 2), (n, 2), (1, n)])
                    WXc = _rap(WXt[:, n0], [(0, 2), (0, 2), (1, n)])
                    WYc = _rap(WYt[:, n0], [(0, 2), (1, n)])
                    nc.vector.tensor_tensor(R, R, L, Alu.subtract)
                    nc.vector.tensor_tensor(R, R, WXc, Alu.mult)
                    nc.vector.tensor_tensor(L, L, R, Alu.add)
                    t0 = _rap(Gt[:, 0, 0], [(n, 2), (1, n)])
                    t1 = _rap(Gt[:, 4, 0], [(n, 2), (1, n)])
                    nc.vector.tensor_tensor(t1, t1, t0, Alu.subtract)
                    nc.vector.tensor_tensor(t1, t1, WYc, Alu.mult)
                    nc.vector.tensor_tensor(
                        _rap(gsb_t[:, 0, n0], [(NPOS2, 2), (1, n)]),
                        t0, t1, Alu.add)

                # last chunk of the previous sample: emitted AFTER this
                # sample's idx math so the next triggers aren't queued
                # behind it on the DVE
                if pend is not None:
                    combine(pend)
                    pend = None

                Gts = [trigger(c2) for c2 in range(NCH)]

                # ------- wx/wy broadcast (after triggers: the combines only
                # ------- need W once the first gather chunk lands) ----------
                w_bf = ip.tile([128, 2, NG], bf16, tag="wbf")   # [p, o, g]
                nc.vector.tensor_copy(
                    w_bf[:], _rap(frac[:, 0, 0], [(1, 2), (2, NG)]))
                ps_t = pst.tile([54, 128], bf16, tag="pst")
                nc.tensor.transpose(ps_t[:], _rap(w_bf[:, 0, 0], [(1, 54)]),
                                    ident[:])
                w_cols = wcp.tile([54, 128], bf16, tag="wcols")
                nc.scalar.copy(w_cols[:], ps_t[:])
                # HBM bounce: [54,128] col-major -> [2, NPOS] row layout
                w_hbm = dp.tile([54, 128], bf16, tag="whbm")
                nc.sync.dma_start(w_hbm[:], w_cols[:])
                w_rows = wcp.tile([2, NPOS2], bf16, tag="wrows")
                nc.vector.memset(w_rows[:, NPOS:NPOS2], 0.0)
                nc.sync.dma_start(
                    w_rows[:, 0:NPOS],
                    w_hbm[:].rearrange("(o g) p -> o (g p)", o=2))
                WY = wp.tile([128, NPOS2], bf16, tag="WY")
                WX = wp.tile([128, NPOS2], bf16, tag="WX")
                for o, Wt in ((0, WY), (1, WX)):
                    for n0 in range(0, NPOS2, 512):
                        n1 = min(n0 + 512, NPOS2)
                        pwt = ps.tile([128, 512], f32, tag="ps")
                        nc.tensor.matmul(pwt[:, 0:n1 - n0], sel[:, o, :],
                                         w_rows[:, n0:n1],
                                         start=True, stop=True)
                        nc.scalar.copy(Wt[:, n0:n1], pwt[:, 0:n1 - n0])

                # dw2/pw2/store of the previous sample overlaps the gather DMA
                if prev is not None:
                    emit_tail(*prev)
                g_sb = gsp.tile([128, 2, NPOS2], bf16, tag="gsb")
                for c2 in range(NCH - 1):
                    combine((Gts[c2], WY, WX, g_sb, c2))
                pend = (Gts[NCH - 1], WY, WX, g_sb, NCH - 1)
                prev = (si, g_sb)
                # prefetch next sample's x during this sample's gather
                if si + 1 < NS:
                    x_lo2 = xp.tile([128, XW], bf16, tag="xlo")
                    x_hi2 = xp.tile([65, XW], bf16, tag="xhi")
                    nc.sync.dma_start(x_lo2[:], x_d[si + 1, 0:128, :])
                    nc.sync.dma_start(x_hi2[:], x_d[si + 1, 128:193, :])
                    xt = (x_lo2, x_hi2)

            combine(pend)
            emit_tail(*prev)

    nc.compile()
    return nc


def _prep_inputs(p):
    x = p['x'].astype(np.float32)
    W1, b1, b_out = _fold_params(p)

    xpad = np.zeros((B, C, PH, PW), np.float32)
    xpad[:, :, 1:PH - 1, 1:PW - 1] = x
    xflat = np.zeros((B, C + 1, XW), np.float32)
    xflat[:, 0:C, 0:NPAD] = xpad.reshape(B, C, NPAD)
    xflat[:, C, :] = 1.0
    xflat = xflat.astype(BF16)

    poff = p['poff_w'].astype(np.float32)          # [2, C, 3, 3]
    wst = np.zeros((C, 18), np.float32)            # col = t*2 + o
    for t in range(9):
        dy, dx = t // 3, t % 3
        for o in range(2):
            wst[:, t * 2 + o] = poff[o, :, dy, dx]
    w1st = np.concatenate([W1.T, wst], axis=1)     # [C, 210]
    w1st_lo = np.ascontiguousarray(w1st[0:128]).astype(BF16)
    w1st_hi = np.zeros((65, C + 18), np.float32)
    w1st_hi[0:64] = w1st[128:192]
    w1st_hi[64, 0:C] = b1
    w1st_hi = w1st_hi.astype(BF16)

    dw2 = p['dw2_w'].astype(np.float32)            # [O, C, 2, 2]
    k2t = np.zeros((4, C, C), np.float32)
    for t in range(4):
        dy, dx = t // 2, t % 2
        k2t[t] = dw2[:, :, dy, dx].T               # [c, o]
    k2t = k2t.astype(BF16)

    pw2t = np.ascontiguousarray(p['pw2_w'].astype(np.float32).T).astype(BF16)
    bout = b_out.reshape(3, 128).astype(np.float32)

    s = np.arange(NPOS, dtype=np.float32)
    ypad = np.floor_divide(np.minimum(s, NPAD - 1), PW)
    xpad_c = np.minimum(s, NPAD - 1) % PW
    base = np.zeros((128, NG, 2), np.float32)
    base[:, :, 0] = (ypad - 1.0 + float(p['poff_b'][0])).reshape(NG, 128).T
    base[:, :, 1] = (xpad_c - 1.0 + float(p['poff_b'][1])).reshape(NG, 128).T

    sel = np.zeros((2, 2, 128), np.float32)
    sel[0, 0, :] = 1.0
    sel[1, 1, :] = 1.0
    sel = sel.astype(BF16)
    ident = np.eye(128, dtype=np.float32).astype(BF16)

    # one-hot shift matrices for the 9-tap sum: out[m] += A[m + d] via
    # lhsT[k, m] = 1 at k = m+d (main) / k = m+d-+128 (block-crossing wrap)
    shifts = np.zeros((17, 128, 128), np.float32)
    shifts[0] = np.eye(128)
    mi = 1
    for t in range(9):
        d = (t // 3 - 1) * PW + (t % 3 - 1)
        if d == 0:
            continue
        for m in range(128):
            k = m + d
            if 0 <= k < 128:
                shifts[mi, k, m] = 1.0
            kw = m + d - 128 if d > 0 else m + d + 128
            if 0 <= kw < 128:
                shifts[mi + 1, kw, m] = 1.0
        mi += 2
    shifts = shifts.astype(BF16)

    # fold matrices: ps_idx[16j+q, 8g+m] = Bt[16m+q, g]
    pfold = np.zeros((8, 128, 128), np.float32)
    for m in range(8):
        for mp in range(128):
            pfold[m, 16 * m + (mp % 16), mp] = 1.0

    shared = dict(w1st_lo=w1st_lo, w1st_hi=w1st_hi,
                  k2t=k2t, pw2t=pw2t, bout=bout, base=base, sel=sel,
                  ident=ident, shifts=shifts, pfold=pfold)
    in_maps = []
    for ci in range(NCORES):
        m = dict(shared)
        m['x'] = np.ascontiguousarray(xflat[ci * NS:(ci + 1) * NS])
        in_maps.append(m)
    return in_maps


def kernel(**inputs):
    from concourse.bass_utils import run_bass_kernel_spmd

    p = {k: np.asarray(v) for k, v in inputs.items()}
    in_maps = _prep_inputs(p)
    nc = build_nc()
    res = run_bass_kernel_spmd(nc, in_maps, core_ids=list(range(NCORES)))
    outs = [res.results[ci]['out'] for ci in range(NCORES)]
    out = np.concatenate([np.asarray(o).astype(np.float32) for o in outs],
                         axis=0)
    return out.reshape(B, CO, HOUT, WOUT)


# revision 16
# speedup vs baseline: 1.3883x; 1.0906x over previous
"""Trainium2 Bass kernel for nn_AFE_78958678770209 (dense_cnn, deformable block).

Pipeline (per sample):
  h   = W1 @ x + b1           (W1 = def_w @ pw1_w @ dw1_w folded on host)
  off = conv3x3(x, poff)      (offsets; bias folded into the base grid)
  g   = bilinear_gather(h, off)
  d2  = conv2x2_s2(g, dw2)    (dw2 bias folded into pw2 bias)
  out = pw2 @ d2 + b_out
Sharding: data-parallel over batch, 32 samples -> 8 cores x 4 samples.

v3 (schedule + instruction-count rework of v2; same gather algorithm):
  - token stripe = [h(s):192 | A(s):18 | z | h(s+1):192+18 junk | z] bf16,
    so ONE strided ACT eviction per block moves both h-pair and the offset
    conv columns (was 2 evictions); taps read A in-place from the stripes.
  - program order per sample: h-blocks -> taps/idx/W -> ALL gather triggers
    -> emit_tail(prev sample) -> combines.  The tail's dw2/pw2 and the next
    sample's h-phase now hide under the ~52us/sample gather DMA window
    (measured floor: 28.7k descriptors x ~116ns / 16 engines).
  - combine: the two v-streams merged into one 3-free-dim op set
    ([128,2,2,NIDX] x-lerp) -> 6 DVE ops per chunk instead of 9.
  - W broadcast evictions split ACT/DVE; 4 SWDGE queues.
"""

import os
import numpy as np
import ml_dtypes

B, C, CO, H, W = 32, 192, 384, 56, 56
PH, PW = H + 2, W + 2              # 58x58 padded raster
NPAD = PH * PW                     # 3364
NPOS = 3456                        # padded to 27*128
NPOS2 = 3456                       # gather stream (== NPOS)
GA_W = 1740                        # g_sb tile A: cols [0, 1740)
GB_OFF = 1682                      # g_sb tile B: cols [1682, 3456)
GB_W = NPOS2 - GB_OFF
NG = NPOS // 128                   # 27 position blocks
NS = 4                             # samples per core
NCORES = 8
XW = 3584                          # x tile width (>= NPOS+1)
TOKB = 1024                        # token stripe bytes (2 pos x 256 bf16)
HOUT, WOUT = 28, 28
NOUT = HOUT * WOUT                 # 784
UT = 392                           # u-tile: 14 output rows x 28

BF16 = ml_dtypes.bfloat16


def _fold_params(p):
    f32 = np.float32
    W1 = (p['def_w'].astype(f32) @ p['pw1_w'].astype(f32) @ p['dw1_w'].astype(f32))
    b1 = (p['def_w'].astype(f32) @ (p['pw1_w'].astype(f32) @ p['dw1_b'].astype(f32)
                                    + p['pw1_b'].astype(f32)) + p['def_b'].astype(f32))
    b_out = p['pw2_w'].astype(f32) @ p['dw2_b'].astype(f32) + p['pw2_b'].astype(f32)
    return W1, b1, b_out


def _sv(ap2d, boff, h, hstride, w, wstride):
    """Strided [P, h, w] view of a 2-dim AP [P, N] at element offset boff."""
    from bass_rust import AP
    return AP(ap2d.tensor, ap2d.offset + boff,
              [list(ap2d.ap[0]), [hstride, h], [wstride, w]])


def _rap(ap, dims):
    """Raw AP with explicit free dims [(stride, n), ...] at ap's offset."""
    from bass_rust import AP
    return AP(ap.tensor, ap.offset, [list(ap.ap[0])] + [list(d) for d in dims])


def build_nc():
    import concourse.bacc as bacc
    import concourse.mybir as mybir
    import concourse.tile as tile

    NQ = int(os.environ.get('KQUEUES', '2'))
    # The LAST combine of each sample is deferred into the next iteration,
    # after the next sample's idx math, so the DVE FIFO never holds up the
    # next gather triggers.  num_idxs must stay <= 1024 (1152 faults the HW
    # exec unit).
    CHUNKS = (896, 896, 896, 768)
    assert sum(CHUNKS) == NPOS2 and all(c % 128 == 0 for c in CHUNKS)
    NCH = len(CHUNKS)
    CH0 = [sum(CHUNKS[:i]) for i in range(NCH)]    # start offsets

    nc = bacc.Bacc("TRN2", target_bir_lowering=False, debug=False,
                   num_swdge_queues=NQ)
    dt = mybir.dt
    Alu = mybir.AluOpType
    f32, bf16, i16 = dt.float32, dt.bfloat16, dt.int16

    # ---------------- DRAM parameters ----------------
    x_d = nc.declare_dram_parameter("x", [NS, C + 1, XW], dt.bfloat16, isOutput=False)
    w1st_lo_d = nc.declare_dram_parameter("w1st_lo", [128, C + 18], dt.bfloat16, isOutput=False)
    w1st_hi_d = nc.declare_dram_parameter("w1st_hi", [65, C + 18], dt.bfloat16, isOutput=False)  # row64=[b1|0]
    k2t_d = nc.declare_dram_parameter("k2t", [4, C, C], dt.bfloat16, isOutput=False)       # [t,(c),(o)]
    pw2t_d = nc.declare_dram_parameter("pw2t", [C, CO], dt.bfloat16, isOutput=False)
    bout_d = nc.declare_dram_parameter("bout", [3, 128], dt.float32, isOutput=False)
    base_d = nc.declare_dram_parameter("base", [128, NG, 2], dt.float32, isOutput=False)
    sel_d = nc.declare_dram_parameter("sel", [2, 2, 128], dt.bfloat16, isOutput=False)
    ident_d = nc.declare_dram_parameter("ident", [128, 128], dt.bfloat16, isOutput=False)
    shifts_d = nc.declare_dram_parameter("shifts", [17, 128, 128], dt.bfloat16,
                                         isOutput=False)
    pfold_d = nc.declare_dram_parameter("pfold", [8, 128, 128], dt.float32,
                                        isOutput=False)
    out_d = nc.declare_dram_parameter("out", [NS, CO, NOUT], dt.bfloat16, isOutput=True)

    TAPS = [(t, dy * PW + dx)
            for t, (dy, dx) in enumerate((dy, dx) for dy in (-1, 0, 1)
                                         for dx in (-1, 0, 1))]

    from contextlib import ExitStack
    with ExitStack() as _stk:
        tc = _stk.enter_context(tile.TileContext(nc))
        _p = lambda **kw: _stk.enter_context(tc.tile_pool(**kw))
        cp = _p(name="const", bufs=1)
        xp = _p(name="x", bufs=1)
        ap_ = _p(name="A", bufs=1)
        ip = _p(name="idx", bufs=2)
        wcp = _p(name="wc", bufs=1)
        wp = _p(name="W", bufs=2)
        gp3 = _p(name="G", bufs=3)
        gp1 = _p(name="Gl", bufs=1)
        tp_ = _p(name="tt", bufs=1)
        sp = _p(name="sc", bufs=1)
        gsp = _p(name="gsb", bufs=1)
        d2p = _p(name="d2", bufs=1)
        op = _p(name="osb", bufs=1)
        dp = _p(name="dram", bufs=1, space="DRAM")
        ps = _p(name="ps", bufs=2, space="PSUM")
        pst = _p(name="pst", bufs=1, space="PSUM")
        psA = _p(name="psA", bufs=1, space="PSUM")
        psH = _p(name="psH", bufs=3, space="PSUM")
        if True:
            # x of sample 0 first so its h-phase starts ASAP, then the
            # weights the first matmuls need, then the rest of the consts.
            x_tiles = []
            x_lo0 = xp.tile([128, XW], bf16, tag="xlo")
            x_hi0 = xp.tile([65, XW], bf16, tag="xhi")
            nc.sync.dma_start(x_lo0[:], x_d[0, 0:128, :])
            nc.sync.dma_start(x_hi0[:], x_d[0, 128:193, :])
            w1st_lo = cp.tile([128, C + 18], bf16)
            nc.sync.dma_start(w1st_lo[:], w1st_lo_d[:])
            w1st_hi = cp.tile([65, C + 18], bf16)
            nc.sync.dma_start(w1st_hi[:], w1st_hi_d[:])
            shifts = cp.tile([128, 17, 128], bf16)
            nc.sync.dma_start(shifts[:], shifts_d[:].rearrange("t k m -> k t m"))
            pfold = cp.tile([128, 8, 128], f32)
            nc.sync.dma_start(pfold[:], pfold_d[:].rearrange("t k m -> k t m"))
            base = cp.tile([128, NG, 2], f32)
            nc.sync.dma_start(base[:], base_d[:])
            sel = cp.tile([2, 2, 128], bf16)
            nc.sync.dma_start(sel[:], sel_d[:])
            ident = cp.tile([128, 128], bf16)
            nc.sync.dma_start(ident[:], ident_d[:])
            k2t_lo = cp.tile([128, 4, C], bf16)
            nc.sync.dma_start(k2t_lo[:],
                              k2t_d[:, 0:128, :].rearrange("t c o -> c t o"))
            k2t_hi = cp.tile([64, 4, C], bf16)
            nc.sync.dma_start(k2t_hi[:],
                              k2t_d[:, 128:192, :].rearrange("t c o -> c t o"))
            pw2t_lo = cp.tile([128, CO], bf16)
            nc.sync.dma_start(pw2t_lo[:], pw2t_d[0:128, :])
            pw2t_hi = cp.tile([64, CO], bf16)
            nc.sync.dma_start(pw2t_hi[:], pw2t_d[128:192, :])
            bout = cp.tile([128, 3], f32)
            nc.sync.dma_start(bout[:], bout_d[:].rearrange("b p -> p b"))

            from concourse import library_config
            nc.gpsimd.load_library(library_config.mlp)

            # token pair buffers (manual ping-pong); stripe =
            # [h(s):192 | A(s):18 | z:46 | h(s+1)+junk:210 | z:46] bf16.
            # Group NG is a permanent zero pad for the shifted tap views.
            tok_bufs = []
            for _tb in range(2):
                _tok = cp.tile([128, NG + 1, 512], bf16, tag=f"tokbuf{_tb}")
                # zero only what must be zero: the tap-pad group NG and the
                # two z-regions of every stripe (cheap; the big full-tile
                # memset serialized ~20us of DVE time at startup).
                nc.vector.memset(_tok[:, NG, :], 0.0)
                nc.vector.memset(_rap(_tok[:, 0, 210], [(512, NG), (1, 46)]), 0.0)
                nc.vector.memset(_rap(_tok[:, 0, 466], [(512, NG), (1, 46)]), 0.0)
                tok_bufs.append(_tok)

            def emit_tail(si, g_sb):
                # ---------------- dw2 (2x2 stride-2) -------------------------
                d2_lo = d2p.tile([128, NOUT], bf16, tag="d2lo")
                d2_hi = d2p.tile([64, NOUT], bf16, tag="d2hi")
                glo = g_sb[:, 0, :]
                ghi = g_sb[0:64, 1, :]
                for obase, osz, dtile in ((0, 128, d2_lo), (128, 64, d2_hi)):
                    for ut in range(2):
                        pd = ps.tile([osz, UT], f32, tag="ps")
                        for t in range(4):
                            dy, dx = t // 2, t % 2
                            boff = PW * (1 + dy) + (1 + dx) + ut * 14 * 2 * PW
                            rhs_lo = _sv(glo, boff, 14, 2 * PW, 28, 2)
                            rhs_hi = _sv(ghi, boff, 14, 2 * PW, 28, 2)
                            nc.tensor.matmul(
                                pd[:], k2t_lo[:, t, obase:obase + osz], rhs_lo,
                                start=(t == 0), stop=False)
                            nc.tensor.matmul(
                                pd[:], k2t_hi[:, t, obase:obase + osz], rhs_hi,
                                start=False, stop=(t == 3))
                        nc.scalar.copy(dtile[:, ut * UT:(ut + 1) * UT], pd[:])
                # ---------------- pw2 ----------------------------------------
                out_sb = op.tile([128, 3, NOUT], bf16, tag="osb")
                for o3 in range(3):
                    osl = slice(o3 * 128, (o3 + 1) * 128)
                    for ut in range(2):
                        usl = slice(ut * UT, (ut + 1) * UT)
                        po = ps.tile([128, UT], f32, tag="ps")
                        nc.tensor.matmul(po[:], pw2t_lo[:, osl],
                                         d2_lo[:, usl], start=True, stop=False)
                        nc.tensor.matmul(po[:], pw2t_hi[:, osl],
                                         d2_hi[:, usl], start=False, stop=True)
                        nc.scalar.add(out_sb[:, o3, usl], po[:],
                                      bout[:, o3:o3 + 1])
                nc.sync.dma_start(
                    out_d[si, :, :].rearrange("(b p) n -> p b n", p=128),
                    out_sb[:])

            prev = None   # (si, g_sb) of the previous sample
            pend = None   # deferred last-chunk combine of the previous sample
            xt = (x_lo0, x_hi0)
            for si in range(NS):
                x_lo, x_hi = xt

                # ------- h pair-tokens + A columns, one eviction/block -------
                tok = tok_bufs[si % 2]
                for g in range(NG):
                    s0 = g * 128
                    ph = psH.tile([128, 420], f32, tag="psH")
                    nc.tensor.matmul(ph[:, 0:210], x_lo[:, s0:s0 + 128],
                                     w1st_lo[:], start=True, stop=False)
                    nc.tensor.matmul(ph[:, 0:210], x_hi[:, s0:s0 + 128],
                                     w1st_hi[:], start=False, stop=True)
                    nc.tensor.matmul(ph[:, 210:420],
                                     x_lo[:, s0 + 1:s0 + 129],
                                     w1st_lo[:], start=True, stop=False)
                    nc.tensor.matmul(ph[:, 210:420],
                                     x_hi[:, s0 + 1:s0 + 129],
                                     w1st_hi[:], start=False, stop=True)
                    # tok[p, g, {0:210, 256:466}] = [h|A](s), [h|A](s+1)
                    nc.scalar.copy(
                        _rap(tok[:, g, 0], [(256, 2), (1, 210)]),
                        _rap(ph[:, 0], [(210, 2), (1, 210)]))

                # ------- 9-tap shifted sum via PE one-hot shift matmuls -------
                # A(s) lives at stripe elems [192,210) of tok slot 0.
                def Arhs(g0, n, co):
                    return _rap(tok[:, g0, 192 + co], [(512, n), (1, 2)])
                ps_off = psA.tile([128, 54], f32, tag="psOff")
                ofull = _rap(ps_off[:, 0], [(2, NG), (1, 2)])
                nc.tensor.matmul(ofull, shifts[:, 0, :], Arhs(0, NG, 8),
                                 start=True, stop=False)
                mi = 1
                for t, d in TAPS:
                    if d == 0:
                        continue
                    co = 2 * t
                    nc.tensor.matmul(ofull, shifts[:, mi, :], Arhs(0, NG, co),
                                     start=False, stop=False)
                    last = (mi + 1 == 16)
                    if d > 0:
                        nc.tensor.matmul(ofull, shifts[:, mi + 1, :],
                                         Arhs(1, NG, co),
                                         start=False, stop=last)
                    else:
                        nc.tensor.matmul(_rap(ps_off[:, 2], [(2, NG - 1), (1, 2)]),
                                         shifts[:, mi + 1, :],
                                         Arhs(0, NG - 1, co),
                                         start=False, stop=last)
                    mi += 2
                acc = ap_.tile([128, NG, 2], f32, tag="acc")
                nc.vector.tensor_copy(acc[:], ofull)

                # ---------------- index math ----------------
                pyx = ap_.tile([128, NG, 2], f32, tag="pyx")
                nc.vector.tensor_tensor(pyx[:], acc[:], base[:], Alu.add)
                nc.vector.tensor_scalar(pyx[:], pyx[:], 0.0, float(H - 1),
                                        Alu.max, Alu.min)
                y0i = ap_.tile([128, NG, 2], dt.int32, tag="y0i")
                nc.vector.tensor_copy(y0i[:], pyx[:])
                icast = ap_.tile([128, NG, 2], f32, tag="icast")
                nc.vector.tensor_copy(icast[:], y0i[:])
                gtt = ap_.tile([128, NG, 2], f32, tag="gtt")
                nc.vector.tensor_tensor(gtt[:], icast[:], pyx[:], Alu.is_gt)
                ifl = ap_.tile([128, NG, 2], f32, tag="ifl")
                nc.vector.tensor_tensor(ifl[:], icast[:], gtt[:], Alu.subtract)
                frac = ap_.tile([128, NG, 2], f32, tag="frac")
                nc.vector.tensor_tensor(frac[:], pyx[:], ifl[:], Alu.subtract)
                # token base id: Bt = 59 + 58*ifl_y + ifl_x
                Bt = ap_.tile([128, NG], f32, tag="Bt")
                nc.vector.tensor_scalar(Bt[:], ifl[:, :, 0], float(PW),
                                        float(PW + 1), Alu.mult, Alu.add)
                nc.vector.tensor_tensor(Bt[:], Bt[:], ifl[:, :, 1], Alu.add)
                # 16-wrap fold via PE one-hot matmuls (fp32, exact ints):
                # ps_idx[16j+q, 8g+m] = Bt[16m+q, g] for all j
                ps_idx = pst.tile([128, 224], f32, tag="pidx")
                for m in range(8):
                    nc.tensor.matmul(_rap(ps_idx[:, m], [(8, NG)]),
                                     pfold[:, m, :], Bt[:],
                                     start=True, stop=True)
                idxs = ip.tile([128, 2, 216], i16, tag="idxs")
                nc.vector.tensor_scalar(idxs[:, 0, 0:216], ps_idx[:, 0:216],
                                        0.0, None, Alu.add)
                nc.vector.tensor_scalar(idxs[:, 1, 0:216], ps_idx[:, 0:216],
                                        float(PW), None, Alu.add)

                # ------- gather triggers + combines, software-pipelined ------
                tok_flat = _rap(tok[:, 0, 0], [(1, NG * 512)])

                def trigger(c2):
                    n0, n = CH0[c2], CHUNKS[c2]
                    pool = gp3 if c2 < NCH - 1 else gp1
                    Gt = pool.tile([128, 8, n], bf16, tag=f"G{n}")
                    for v in range(2):
                        nc.gpsimd.dma_gather(
                            Gt[:, 4 * v:4 * (v + 1), :],
                            tok_flat,
                            idxs[:, v, n0 // 16:(n0 + n) // 16],
                            num_idxs=n, num_idxs_reg=n,
                            elem_size=512, transpose=True,
                            queue_num=(c2 * 2 + v) % NQ,
                            sbuf_tokens_per_rank=128,
                            sbuf_free_dim_per_rank=TOKB)
                    return Gt

                def combine(ctx):
                    Gt, WYt, WXt, g_sb, c2 = ctx
                    n0, n = CH0[c2], CHUNKS[c2]
                    # corners: [v0: L(2) R(2) | v1: L(2) R(2)] groups of n.
                    # All lerps run in-place inside Gt (no scratch tiles):
                    #   R := (R - L) * wx;  L := L + R   (x-lerp, both v)
                    #   t1 := (t1 - t0) * wy; out := t0 + t1   (y-lerp)
                    L = _rap(Gt[:, 0, 0], [(4 * n, 2), (n, 2), (1, n)])
                    R = _rap(Gt[:, 2, 0], [(4 * n, 2), (n, 2), (1, n)])
                    WXc = _rap(WXt[:, n0], [(0, 2), (0, 2), (1, n)])
                    WYc = _rap(WYt[:, n0], [(0, 2), (1, n)])
                    nc.vector.tensor_tensor(R, R, L, Alu.subtract)
                    nc.vector.tensor_tensor(R, R, WXc, Alu.mult)
                    nc.vector.tensor_tensor(L, L, R, Alu.add)
                    t1 = _rap(Gt[:, 4, 0], [(n, 2), (1, n)])
                    nc.vector.tensor_tensor(
                        t1, t1, _rap(Gt[:, 0, 0], [(n, 2), (1, n)]),
                        Alu.subtract)
                    nc.vector.tensor_tensor(t1, t1, WYc, Alu.mult)
                    nc.vector.tensor_tensor(
                        _rap(g_sb[:, 0, n0], [(NPOS2, 2), (1, n)]),
                        _rap(Gt[:, 0, 0], [(n, 2), (1, n)]), t1, Alu.add)

                # last chunk of the previous sample: emitted AFTER this
                # sample's idx math so the next triggers aren't queued
                # behind it on the DVE
                if pend is not None:
                    combine(pend)
                    pend = None

                Gts = [trigger(c2) for c2 in range(NCH)]

                # ------- wx/wy broadcast (after triggers: the combines only
                # ------- need W once the first gather chunk lands) ----------
                w_bf = ip.tile([128, 2, NG], bf16, tag="wbf")   # [p, o, g]
                nc.vector.tensor_copy(
                    w_bf[:], _rap(frac[:, 0, 0], [(1, 2), (2, NG)]))
                ps_t = pst.tile([54, 128], bf16, tag="pst")
                nc.tensor.transpose(ps_t[:], _rap(w_bf[:, 0, 0], [(1, 54)]),
                                    ident[:])
                w_cols = wcp.tile([54, 128], bf16, tag="wcols")
                nc.scalar.copy(w_cols[:], ps_t[:])
                # HBM bounce: [54,128] col-major -> [2, NPOS] row layout
                w_hbm = dp.tile([54, 128], bf16, tag="whbm")
                nc.sync.dma_start(w_hbm[:], w_cols[:])
                w_rows = wcp.tile([2, NPOS2], bf16, tag="wrows")
                nc.sync.dma_start(
                    w_rows[:, 0:NPOS],
                    w_hbm[:].rearrange("(o g) p -> o (g p)", o=2))
                WY = wp.tile([128, NPOS2], bf16, tag="WY")
                WX = wp.tile([128, NPOS2], bf16, tag="WX")
                for o, Wt in ((0, WY), (1, WX)):
                    for n0 in range(0, NPOS2, 512):
                        n1 = min(n0 + 512, NPOS2)
                        pwt = ps.tile([128, 512], f32, tag="ps")
                        nc.tensor.matmul(pwt[:, 0:n1 - n0], sel[:, o, :],
                                         w_rows[:, n0:n1],
                                         start=True, stop=True)
                        nc.scalar.copy(Wt[:, n0:n1], pwt[:, 0:n1 - n0])

                # dw2/pw2/store of the previous sample overlaps the gather DMA
                if prev is not None:
                    emit_tail(*prev)
                g_sb = gsp.tile([128, 2, NPOS2], bf16, tag="gsb")
                for c2 in range(NCH - 1):
                    combine((Gts[c2], WY, WX, g_sb, c2))
                pend = (Gts[NCH - 1], WY, WX, g_sb, NCH - 1)
                prev = (si, g_sb)
                # prefetch next sample's x during this sample's gather
                if si + 1 < NS:
                    x_lo2 = xp.tile([128, XW], bf16, tag="xlo")
                    x_hi2 = xp.tile([65, XW], bf16, tag="xhi")
                    nc.sync.dma_start(x_lo2[:], x_d[si + 1, 0:128, :])
                    nc.sync.dma_start(x_hi2[:], x_d[si + 1, 128:193, :])
                    xt = (x_lo2, x_hi2)

            combine(pend)
            emit_tail(*prev)

    nc.compile()
    return nc


def _prep_inputs(p):
    x = p['x'].astype(np.float32)
    W1, b1, b_out = _fold_params(p)

    xpad = np.zeros((B, C, PH, PW), np.float32)
    xpad[:, :, 1:PH - 1, 1:PW - 1] = x
    xflat = np.zeros((B, C + 1, XW), np.float32)
    xflat[:, 0:C, 0:NPAD] = xpad.reshape(B, C, NPAD)
    xflat[:, C, :] = 1.0
    xflat = xflat.astype(BF16)

    poff = p['poff_w'].astype(np.float32)          # [2, C, 3, 3]
    wst = np.zeros((C, 18), np.float32)            # col = t*2 + o
    for t in range(9):
        dy, dx = t // 3, t % 3
        for o in range(2):
            wst[:, t * 2 + o] = poff[o, :, dy, dx]
    w1st = np.concatenate([W1.T, wst], axis=1)     # [C, 210]
    w1st_lo = np.ascontiguousarray(w1st[0:128]).astype(BF16)
    w1st_hi = np.zeros((65, C + 18), np.float32)
    w1st_hi[0:64] = w1st[128:192]
    w1st_hi[64, 0:C] = b1
    w1st_hi = w1st_hi.astype(BF16)

    dw2 = p['dw2_w'].astype(np.float32)            # [O, C, 2, 2]
    k2t = np.zeros((4, C, C), np.float32)
    for t in range(4):
        dy, dx = t // 2, t % 2
        k2t[t] = dw2[:, :, dy, dx].T               # [c, o]
    k2t = k2t.astype(BF16)

    pw2t = np.ascontiguousarray(p['pw2_w'].astype(np.float32).T).astype(BF16)
    bout = b_out.reshape(3, 128).astype(np.float32)

    s = np.arange(NPOS, dtype=np.float32)
    ypad = np.floor_divide(np.minimum(s, NPAD - 1), PW)
    xpad_c = np.minimum(s, NPAD - 1) % PW
    base = np.zeros((128, NG, 2), np.float32)
    base[:, :, 0] = (ypad - 1.0 + float(p['poff_b'][0])).reshape(NG, 128).T
    base[:, :, 1] = (xpad_c - 1.0 + float(p['poff_b'][1])).reshape(NG, 128).T

    sel = np.zeros((2, 2, 128), np.float32)
    sel[0, 0, :] = 1.0
    sel[1, 1, :] = 1.0
    sel = sel.astype(BF16)
    ident = np.eye(128, dtype=np.float32).astype(BF16)

    # one-hot shift matrices for the 9-tap sum: out[m] += A[m + d] via
    # lhsT[k, m] = 1 at k = m+d (main) / k = m+d-+128 (block-crossing wrap)
    shifts = np.zeros((17, 128, 128), np.float32)
    shifts[0] = np.eye(128)
    mi = 1
    for t in range(9):
        d = (t // 3 - 1) * PW + (t % 3 - 1)
        if d == 0:
            continue
        for m in range(128):
            k = m + d
            if 0 <= k < 128:
                shifts[mi, k, m] = 1.0
            kw = m + d - 128 if d > 0 else m + d + 128
            if 0 <= kw < 128:
                shifts[mi + 1, kw, m] = 1.0
        mi += 2
    shifts = shifts.astype(BF16)

    # fold matrices: ps_idx[16j+q, 8g+m] = Bt[16m+q, g]
    pfold = np.zeros((8, 128, 128), np.float32)
    for m in range(8):
        for mp in range(128):
            pfold[m, 16 * m + (mp % 16), mp] = 1.0

    shared = dict(w1st_lo=w1st_lo, w1st_hi=w1st_hi,
                  k2t=k2t, pw2t=pw2t, bout=bout, base=base, sel=sel,
                  ident=ident, shifts=shifts, pfold=pfold)
    in_maps = []
    for ci in range(NCORES):
        m = dict(shared)
        m['x'] = np.ascontiguousarray(xflat[ci * NS:(ci + 1) * NS])
        in_maps.append(m)
    return in_maps


def kernel(**inputs):
    from concourse.bass_utils import run_bass_kernel_spmd

    p = {k: np.asarray(v) for k, v in inputs.items()}
    in_maps = _prep_inputs(p)
    nc = build_nc()
    res = run_bass_kernel_spmd(nc, in_maps, core_ids=list(range(NCORES)))
    outs = [res.results[ci]['out'] for ci in range(NCORES)]
    out = np.concatenate([np.asarray(o).astype(np.float32) for o in outs],
                         axis=0)
    return out.reshape(B, CO, HOUT, WOUT)
